# revision 24
# baseline (speedup 1.0000x reference)
"""Heat-kernel graph diffusion on 8 Trainium2 NeuronCores.

Computes out = expm(-t*L) @ x for a graph Laplacian L [2048,2048] and node
features x [2048,512], t scalar.

Method: Chebyshev expansion of exp(-t*lam) on [0, lam_b] applied to the
action on x (no dense expm):
    out = sum_k c_k T_k(M) x,   M = (2/lam_b) L - I,
    c_0 = e^{-a} I_0(a), c_k = 2 e^{-a} (-1)^k I_k(a),  a = t*lam_b/2,
with lam_b = 2*max(diag(L)) (Gershgorin bound for a Laplacian; always
>= lam_max). K ~ 20 terms for t=0.5. Bessel I_k via Miller's backward
recurrence (pure numpy, no scipy).

Sharding: x column-sharded 8 ways (64 channels/core), L replicated; the
recurrence is embarrassingly parallel across channels - no collectives.

Device kernel (per core, natural layout [node, ch]):
  - L is exactly representable in bf16 (entries are multiples of 0.5 < 256),
    so it is passed pre-cast to bf16 and used as 128x128 stationary matmul
    weights (full PE array, 1 cyc/row). If a pathological L is not bf16-exact,
    a second bf16 matrix L_lo = L - bf16(L) is also multiplied in.
  - fp32 state y is split per term into bf16 hi+lo halves, concatenated as a
    [128, 128] moving operand; PSUM accumulates z_hi|z_lo in fp32.
  - Chebyshev recurrence y_next = 2a*(L y) - 2 y - y_prev and accumulation
    run in fp32 on the Vector/Scalar engines.
Measured end-to-end relative error vs the fp64 reference path: ~3e-5.
"""

import functools
import math

import numpy as np
import ml_dtypes

import concourse.bacc as bacc
import concourse.mybir as mybir
import concourse.tile as tile
from concourse.bass_utils import run_bass_kernel_spmd

N = 2048
D = 512
NCORES = 8
DSH = D // NCORES      # 64 channels per core
P = 128                # partitions
KB = N // P            # 16 contraction blocks
IB = N // P            # 16 output-row blocks
COEF_TOL = 3e-6
KMAX = 280

BF16 = np.dtype(ml_dtypes.bfloat16)


def _bessel_ive(nmax, a):
    """e^{-a} I_k(a), k=0..nmax, via Miller's backward recurrence (float64)."""
    if a < 1e-12:
        out = np.zeros(nmax + 1)
        out[0] = 1.0
        return out
    m = int(max(nmax, a) + 40 + 2 * math.sqrt(max(nmax, a)))
    r = np.zeros(m + 2)
    r[m] = 1e-300
    for k in range(m, 0, -1):
        r[k - 1] = r[k + 1] + (2.0 * k / a) * r[k]
        if r[k - 1] > 1e250:
            r /= r[k - 1]
    s = r[0] + 2.0 * np.sum(r[1:m + 1])
    return r[: nmax + 1] / s


def _cheb_coeffs(t, lam_b, tol=COEF_TOL, kcap=KMAX):
    a = t * lam_b / 2.0
    iv = _bessel_ive(kcap, a)
    c = np.empty(kcap + 1)
    c[0] = iv[0]
    c[1:] = 2.0 * iv[1:] * ((-1.0) ** np.arange(1, kcap + 1))
    keep = np.nonzero(np.abs(c) > tol)[0]
    K = max(1, int(keep[-1]) if len(keep) else 1)
    return c[: K + 1]


@functools.lru_cache(maxsize=4)
def _build(coeffs_key, alpha, use_llo):
    """Compile the per-core NEFF. coeffs_key: tuple of per-term float coeffs."""
    c = np.array(coeffs_key, dtype=np.float64)
    K = len(c) - 1
    f32 = mybir.dt.float32
    bf16 = mybir.dt.bfloat16

    nc = bacc.Bacc("TRN2", target_bir_lowering=False, debug=False,
                   num_devices=NCORES)
    L_d = nc.dram_tensor("L", [N, N], bf16, kind="ExternalInput").ap()
    Llo_d = None
    if use_llo:
        Llo_d = nc.dram_tensor("Llo", [N, N], bf16, kind="ExternalInput").ap()
    x_d = nc.dram_tensor("x", [N, DSH], f32, kind="ExternalInput").ap()
    o_d = nc.dram_tensor("out", [N, DSH], f32, kind="ExternalOutput").ap()

    with tile.TileContext(nc) as tc:
        with tc.tile_pool(name="big", bufs=1) as big, \
             tc.tile_pool(name="state", bufs=1) as state, \
             tc.tile_pool(name="psum", bufs=2, space="PSUM") as psum:
            x_sb = state.tile([P, KB, DSH], f32, tag="x")
            nc.sync.dma_start(out=x_sb, in_=x_d.rearrange("(k p) c -> p k c", p=P))
            L_sb = big.tile([P, KB, N], bf16, tag="L")
            for kb in range(KB):
                eng = nc.sync if kb % 2 == 0 else nc.gpsimd
                eng.dma_start(out=L_sb[:, kb, :],
                              in_=L_d[kb * P:(kb + 1) * P, :])
            if use_llo:
                Llo_sb = big.tile([P, KB, N], bf16, tag="Llo")
                for kb in range(KB):
                    nc.sync.dma_start(out=Llo_sb[:, kb, :],
                                      in_=Llo_d[kb * P:(kb + 1) * P, :])

            # state buffers (rotating) + accumulator + scratch
            ys = [state.tile([P, KB, DSH], f32, tag=f"y{i}", name=f"y{i}")
                  for i in range(3)]
            acc = state.tile([P, KB, DSH], f32, tag="acc")
            zh = state.tile([P, KB, DSH], f32, tag="zh")
            zh2 = state.tile([P, KB, DSH], f32, tag="zh2")
            u = state.tile([P, KB, DSH], f32, tag="u")
            u2 = state.tile([P, KB, DSH], f32, tag="u2")
            q = state.tile([P, KB, DSH], f32, tag="q")
            w_acc = state.tile([P, KB, DSH], f32, tag="w_acc")
            # double-buffered hi|lo moving operand: term k reads cats[k%2],
            # term k's splits write cats[(k+1)%2] (no WAR with own matmuls)
            cats = [state.tile([P, KB, 2 * DSH], bf16, tag=f"cat{i}",
                               name=f"cat{i}") for i in range(2)]

            sub = mybir.AluOpType.subtract
            add = mybir.AluOpType.add
            mult = mybir.AluOpType.mult

            SL = 4                 # slicing of the early chain + phase split
            SKB = KB // SL
            SLB = 8                # fine slicing of the boundary chain
            SKB_B = KB // SLB

            def split_into_cat(src, cat, sl):
                """cat slice <- [bf16(src) | bf16(src - hi)], all on DVE
                (fp32 copy runs 2x there and avoids cross-engine latency)"""
                hi = cat[:, sl, 0:DSH]
                lo = cat[:, sl, DSH:2 * DSH]
                nc.vector.tensor_copy(out=hi, in_=src[:, sl])
                nc.vector.scalar_tensor_tensor(out=lo, in0=hi, scalar=-1.0,
                                               in1=src[:, sl], op0=mult, op1=add)

            # y0 = x; acc = c0 * x
            nc.vector.tensor_copy(out=ys[0], in_=x_sb)
            nc.vector.tensor_scalar_mul(acc, x_sb, float(c[0]))
            for s in range(SL):
                split_into_cat(ys[0], cats[1], slice(s * SKB, (s + 1) * SKB))

            for k in range(1, K + 1):
                scale = float(2.0 * alpha) if k >= 2 else float(alpha)
                cat_r = cats[k % 2]
                cat_w = cats[(k + 1) % 2]
                y_cur = ys[(k - 1) % 3]
                y_next = ys[k % 3]
                # two PSUM regions, each a contiguous accumulation group:
                # ps1 sums kb 0..PH-1 (ready 75% into the term), ps2 sums the
                # tail kb. The next term's phase-1 matmuls only need ps1's
                # readers done, so the vector chain never stalls the PE.
                ps = psum.tile([P, IB, 2 * DSH], f32, tag="ps", bufs=1)
                ps2 = psum.tile([P, IB, 2 * DSH], f32, tag="ps2", bufs=1)

                # q = -2*y_cur - y_prev (k>=2) or -y0 (k==1): ready before PSUM,
                # overlaps the matmul sweep (coarse slices: fewer, bigger ops)
                for s in range(2):
                    sl = slice(s * (KB // 2), (s + 1) * (KB // 2))
                    if k == 1:
                        nc.vector.tensor_scalar_mul(q[:, sl], y_cur[:, sl], -1.0)
                    else:
                        y_prev = ys[(k - 2) % 3]
                        nc.vector.scalar_tensor_tensor(
                            out=q[:, sl], in0=y_cur[:, sl], scalar=-2.0,
                            in1=y_prev[:, sl], op0=mult, op1=sub)

                # two-phase contraction: phase 1 (kb 0..PH-1) only needs the
                # early cat slices, so it can start while the previous term's
                # tail slices are still in the vector chain -> no PE bubble
                PH = KB - SKB
                for tgt, lo_kb, hi_kb in ((ps, 0, PH), (ps2, PH, KB)):
                    for ib in range(IB):
                        for kb in range(lo_kb, hi_kb):
                            nc.tensor.matmul(
                                tgt[:, ib, :],
                                L_sb[:, kb, ib * P:(ib + 1) * P],
                                cat_r[:, kb, :],
                                start=(kb == lo_kb),
                                stop=(kb == hi_kb - 1),
                            )
                            if use_llo:
                                # correction term L_lo @ y_hi summed into the
                                # hi half (the chain adds both halves anyway)
                                nc.tensor.matmul(
                                    tgt[:, ib, 0:DSH],
                                    Llo_sb[:, kb, ib * P:(ib + 1) * P],
                                    cat_r[:, kb, 0:DSH],
                                    start=False,
                                    stop=(kb == hi_kb - 1),
                                    skip_group_check=True,
                                )

                for s in range(SL):
                    sl = slice(s * SKB, (s + 1) * SKB)
                    # early part (only needs ps, ready 75% into the term):
                    # u = scale*(ps.hi + ps.lo) + q
                    nc.scalar.mul(zh[:, sl], ps[:, sl, 0:DSH], scale)
                    nc.vector.scalar_tensor_tensor(
                        out=u[:, sl], in0=ps[:, sl, DSH:2 * DSH], scalar=scale,
                        in1=zh[:, sl], op0=mult, op1=add)
                    nc.vector.tensor_add(out=u[:, sl], in0=u[:, sl],
                                         in1=q[:, sl])
                # boundary chain: fine slices chase ph2's ib-completion wave;
                # per slice: y_next = u + scale*(ps2.hi+ps2.lo), then bf16 split
                for s in range(SLB):
                    sl = slice(s * SKB_B, (s + 1) * SKB_B)
                    nc.scalar.mul(zh2[:, sl], ps2[:, sl, 0:DSH], scale)
                    nc.vector.scalar_tensor_tensor(
                        out=u2[:, sl], in0=ps2[:, sl, DSH:2 * DSH], scalar=scale,
                        in1=zh2[:, sl], op0=mult, op1=add)
                    nc.vector.tensor_add(out=y_next[:, sl], in0=u[:, sl],
                                         in1=u2[:, sl])
                    if k < K:
                        split_into_cat(y_next, cat_w, sl)
                # acc += c_k * y_next (off critical path, after the splits)
                for s in range(2):
                    sl = slice(s * (KB // 2), (s + 1) * (KB // 2))
                    nc.vector.scalar_tensor_tensor(
                        out=acc[:, sl], in0=y_next[:, sl], scalar=float(c[k]),
                        in1=acc[:, sl], op0=mult, op1=add)

            nc.sync.dma_start(out=o_d.rearrange("(k p) c -> p k c", p=P), in_=acc)

    nc.compile()
    return nc


def kernel(x, L, t):
    x = np.ascontiguousarray(np.asarray(x, dtype=np.float32))
    L = np.ascontiguousarray(np.asarray(L, dtype=np.float32))
    tv = float(max(float(np.asarray(t, dtype=np.float32)), 1e-8))
    assert x.shape == (N, D) and L.shape == (N, N)

    lam_b = max(2.0 * float(np.diagonal(L).max()), 1e-6)
    alpha = 2.0 / lam_b
    c = _cheb_coeffs(tv, lam_b)

    L_hi = L.astype(BF16)
    L_res = L - L_hi.astype(np.float32)
    use_llo = bool(np.any(L_res != 0.0))

    nc = _build(tuple(float(v) for v in c), float(alpha), use_llo)

    in_maps = []
    for core in range(NCORES):
        m = {"L": L_hi, "x": np.ascontiguousarray(x[:, core * DSH:(core + 1) * DSH])}
        if use_llo:
            m["Llo"] = L_res.astype(BF16)
        in_maps.append(m)

    res = run_bass_kernel_spmd(nc, in_maps, core_ids=list(range(NCORES)))
    out = np.empty((N, D), dtype=np.float32)
    for core in range(NCORES):
        out[:, core * DSH:(core + 1) * DSH] = res.results[core]["out"]
    kernel.last_exec_time_ns = res.exec_time_ns
    kernel.last_results = res
    return out


kernel.last_exec_time_ns = None
kernel.last_results = None


# revision 25
# speedup vs baseline: 1.0092x; 1.0092x over previous
"""Heat-kernel graph diffusion on 8 Trainium2 NeuronCores.

Computes out = expm(-t*L) @ x for a graph Laplacian L [2048,2048] and node
features x [2048,512], t scalar.

Method: Chebyshev expansion of exp(-t*lam) on [0, lam_b] applied to the
action on x (no dense expm):
    out = sum_k c_k T_k(M) x,   M = (2/lam_b) L - I,
    c_0 = e^{-a} I_0(a), c_k = 2 e^{-a} (-1)^k I_k(a),  a = t*lam_b/2,
with lam_b = 2*max(diag(L)) (Gershgorin bound for a Laplacian; always
>= lam_max). K ~ 20 terms for t=0.5. Bessel I_k via Miller's backward
recurrence (pure numpy, no scipy).

Sharding: x column-sharded 8 ways (64 channels/core), L replicated; the
recurrence is embarrassingly parallel across channels - no collectives.

Device kernel (per core, natural layout [node, ch]):
  - L is exactly representable in bf16 (entries are multiples of 0.5 < 256),
    so it is passed pre-cast to bf16 and used as 128x128 stationary matmul
    weights (full PE array, 1 cyc/row). If a pathological L is not bf16-exact,
    a second bf16 matrix L_lo = L - bf16(L) is also multiplied in.
  - fp32 state y is split per term into bf16 hi+lo halves, concatenated as a
    [128, 128] moving operand; PSUM accumulates z_hi|z_lo in fp32.
  - Chebyshev recurrence y_next = 2a*(L y) - 2 y - y_prev and accumulation
    run in fp32 on the Vector/Scalar engines.
Measured end-to-end relative error vs the fp64 reference path: ~3e-5.
"""

import functools
import math

import numpy as np
import ml_dtypes

import concourse.bacc as bacc
import concourse.mybir as mybir
import concourse.tile as tile
from concourse.bass_utils import run_bass_kernel_spmd

N = 2048
D = 512
NCORES = 8
DSH = D // NCORES      # 64 channels per core
P = 128                # partitions
KB = N // P            # 16 contraction blocks
IB = N // P            # 16 output-row blocks
COEF_TOL = 3e-6
KMAX = 280

BF16 = np.dtype(ml_dtypes.bfloat16)


def _bessel_ive(nmax, a):
    """e^{-a} I_k(a), k=0..nmax, via Miller's backward recurrence (float64)."""
    if a < 1e-12:
        out = np.zeros(nmax + 1)
        out[0] = 1.0
        return out
    m = int(max(nmax, a) + 40 + 2 * math.sqrt(max(nmax, a)))
    r = np.zeros(m + 2)
    r[m] = 1e-300
    for k in range(m, 0, -1):
        r[k - 1] = r[k + 1] + (2.0 * k / a) * r[k]
        if r[k - 1] > 1e250:
            r /= r[k - 1]
    s = r[0] + 2.0 * np.sum(r[1:m + 1])
    return r[: nmax + 1] / s


def _cheb_coeffs(t, lam_b, tol=COEF_TOL, kcap=KMAX):
    a = t * lam_b / 2.0
    iv = _bessel_ive(kcap, a)
    c = np.empty(kcap + 1)
    c[0] = iv[0]
    c[1:] = 2.0 * iv[1:] * ((-1.0) ** np.arange(1, kcap + 1))
    keep = np.nonzero(np.abs(c) > tol)[0]
    K = max(1, int(keep[-1]) if len(keep) else 1)
    return c[: K + 1]


@functools.lru_cache(maxsize=4)
def _build(coeffs_key, alpha, use_llo):
    """Compile the per-core NEFF. coeffs_key: tuple of per-term float coeffs."""
    c = np.array(coeffs_key, dtype=np.float64)
    K = len(c) - 1
    f32 = mybir.dt.float32
    bf16 = mybir.dt.bfloat16

    nc = bacc.Bacc("TRN2", target_bir_lowering=False, debug=False,
                   num_devices=NCORES)
    L_d = nc.dram_tensor("L", [N, N], bf16, kind="ExternalInput").ap()
    Llo_d = None
    if use_llo:
        Llo_d = nc.dram_tensor("Llo", [N, N], bf16, kind="ExternalInput").ap()
    x_d = nc.dram_tensor("x", [N, DSH], f32, kind="ExternalInput").ap()
    o_d = nc.dram_tensor("out", [N, DSH], f32, kind="ExternalOutput").ap()

    with tile.TileContext(nc) as tc:
        with tc.tile_pool(name="big", bufs=1) as big, \
             tc.tile_pool(name="state", bufs=1) as state, \
             tc.tile_pool(name="psum", bufs=2, space="PSUM") as psum:
            x_sb = state.tile([P, KB, DSH], f32, tag="x")
            nc.sync.dma_start(out=x_sb, in_=x_d.rearrange("(k p) c -> p k c", p=P))
            L_sb = big.tile([P, KB, N], bf16, tag="L")
            for kb in range(KB):
                eng = nc.sync if kb % 2 == 0 else nc.gpsimd
                eng.dma_start(out=L_sb[:, kb, :],
                              in_=L_d[kb * P:(kb + 1) * P, :])
            if use_llo:
                Llo_sb = big.tile([P, KB, N], bf16, tag="Llo")
                for kb in range(KB):
                    nc.sync.dma_start(out=Llo_sb[:, kb, :],
                                      in_=Llo_d[kb * P:(kb + 1) * P, :])

            # state buffers (rotating) + accumulator + scratch
            ys = [state.tile([P, KB, DSH], f32, tag=f"y{i}", name=f"y{i}")
                  for i in range(3)]
            acc = state.tile([P, KB, DSH], f32, tag="acc")
            zh = state.tile([P, KB, DSH], f32, tag="zh")
            zh2 = state.tile([P, KB, DSH], f32, tag="zh2")
            u = state.tile([P, KB, DSH], f32, tag="u")
            u2 = state.tile([P, KB, DSH], f32, tag="u2")
            q = state.tile([P, KB, DSH], f32, tag="q")
            w_acc = state.tile([P, KB, DSH], f32, tag="w_acc")
            # double-buffered hi|lo moving operand: term k reads cats[k%2],
            # term k's splits write cats[(k+1)%2] (no WAR with own matmuls)
            cats = [state.tile([P, KB, 2 * DSH], bf16, tag=f"cat{i}",
                               name=f"cat{i}") for i in range(2)]

            sub = mybir.AluOpType.subtract
            add = mybir.AluOpType.add
            mult = mybir.AluOpType.mult

            # chain slices over ib blocks: big early, tiny at the tail so the
            # boundary chain (last blocks) is short
            SLICES = [(0, 4), (4, 8), (8, 12), (12, 14), (14, 15), (15, 16)]

            def split_into_cat(src, cat, sl, sc):
                """cat slice <- [bf16(sc*src) | bf16(sc*src - hi)]"""
                hi = cat[:, sl, 0:DSH]
                lo = cat[:, sl, DSH:2 * DSH]
                nc.scalar.mul(hi, src[:, sl], sc)
                nc.vector.scalar_tensor_tensor(out=lo, in0=src[:, sl],
                                               scalar=sc, in1=hi,
                                               op0=mult, op1=sub)

            # y0 = x; acc = c0 * x; cat_1 = split(alpha * x)
            nc.vector.tensor_copy(out=ys[0], in_=x_sb)
            nc.vector.tensor_scalar_mul(acc, x_sb, float(c[0]))
            for a, b in SLICES:
                split_into_cat(ys[0], cats[1], slice(a, b), float(alpha))

            for k in range(1, K + 1):
                # cat_k carries sc_k*y_k with sc_k = alpha (k=0) else 2*alpha,
                # so ps accumulates sc_k * L y_k directly and the recurrence is
                # y_next = ps.hi + ps.lo + q with q = -2y - y_prev (or -y0)
                sc_next = float(2.0 * alpha)
                cat_r = cats[k % 2]
                cat_w = cats[(k + 1) % 2]
                y_cur = ys[(k - 1) % 3]
                y_next = ys[k % 3]
                ps = psum.tile([P, IB, 2 * DSH], f32, tag="ps")

                # q overlaps the matmul sweep (coarse slices)
                for s in range(2):
                    sl = slice(s * (KB // 2), (s + 1) * (KB // 2))
                    if k == 1:
                        nc.vector.tensor_scalar_mul(q[:, sl], y_cur[:, sl], -1.0)
                    else:
                        y_prev = ys[(k - 2) % 3]
                        nc.vector.scalar_tensor_tensor(
                            out=q[:, sl], in0=y_cur[:, sl], scalar=-2.0,
                            in1=y_prev[:, sl], op0=mult, op1=sub)

                # ib-outer sweep: region ib's full contraction completes
                # progressively, so the chain publishes cat blocks 0..11 before
                # the term ends; only the last blocks ride the boundary
                for ib in range(IB):
                    for kb in range(KB):
                        nc.tensor.matmul(
                            ps[:, ib, :],
                            L_sb[:, kb, ib * P:(ib + 1) * P],
                            cat_r[:, kb, :],
                            start=(kb == 0),
                            stop=(kb == KB - 1 and not use_llo),
                        )
                    if use_llo:
                        # correction L_lo @ (sc*y_hi) summed into the hi half
                        for kb in range(KB):
                            nc.tensor.matmul(
                                ps[:, ib, 0:DSH],
                                Llo_sb[:, kb, ib * P:(ib + 1) * P],
                                cat_r[:, kb, 0:DSH],
                                start=False,
                                stop=(kb == KB - 1),
                            )

                for a, b in SLICES:
                    sl = slice(a, b)
                    # y_next = ps.hi + ps.lo + q (two PSUM-sourced stt ops)
                    nc.vector.scalar_tensor_tensor(
                        out=u[:, sl], in0=ps[:, sl, 0:DSH], scalar=1.0,
                        in1=q[:, sl], op0=mult, op1=add)
                    nc.vector.scalar_tensor_tensor(
                        out=y_next[:, sl], in0=ps[:, sl, DSH:2 * DSH],
                        scalar=1.0, in1=u[:, sl], op0=mult, op1=add)
                    if k < K:
                        split_into_cat(y_next, cat_w, sl, sc_next)
                # acc += c_k * y_next (off critical path, after the splits)
                for s in range(2):
                    sl = slice(s * (KB // 2), (s + 1) * (KB // 2))
                    nc.vector.scalar_tensor_tensor(
                        out=acc[:, sl], in0=y_next[:, sl], scalar=float(c[k]),
                        in1=acc[:, sl], op0=mult, op1=add)

            nc.sync.dma_start(out=o_d.rearrange("(k p) c -> p k c", p=P), in_=acc)

    nc.compile()
    return nc


def kernel(x, L, t):
    x = np.ascontiguousarray(np.asarray(x, dtype=np.float32))
    L = np.ascontiguousarray(np.asarray(L, dtype=np.float32))
    tv = float(max(float(np.asarray(t, dtype=np.float32)), 1e-8))
    assert x.shape == (N, D) and L.shape == (N, N)

    lam_b = max(2.0 * float(np.diagonal(L).max()), 1e-6)
    alpha = 2.0 / lam_b
    c = _cheb_coeffs(tv, lam_b)

    L_hi = L.astype(BF16)
    L_res = L - L_hi.astype(np.float32)
    use_llo = bool(np.any(L_res != 0.0))

    nc = _build(tuple(float(v) for v in c), float(alpha), use_llo)

    in_maps = []
    for core in range(NCORES):
        m = {"L": L_hi, "x": np.ascontiguousarray(x[:, core * DSH:(core + 1) * DSH])}
        if use_llo:
            m["Llo"] = L_res.astype(BF16)
        in_maps.append(m)

    res = run_bass_kernel_spmd(nc, in_maps, core_ids=list(range(NCORES)))
    out = np.empty((N, D), dtype=np.float32)
    for core in range(NCORES):
        out[:, core * DSH:(core + 1) * DSH] = res.results[core]["out"]
    kernel.last_exec_time_ns = res.exec_time_ns
    kernel.last_results = res
    return out


kernel.last_exec_time_ns = None
kernel.last_results = None
